# revision 2
# baseline (speedup 1.0000x reference)
"""Trainium2 Bass kernel for CrossLoRALinear:
    y = x @ W_base^T + b_base + ((x @ A^T) @ B^T) * SCALE

Strategy (8 NeuronCores, SPMD):
  - Data-parallel shard of the 4*4096=16384 tokens: 2048 tokens/core.
  - Replicate W_base/A/B/b_base.
  - On device, fold the rank-16 LoRA update into the weights once:
        W_effT[k,o] = W_baseT[k,o] + SCALE * (A^T @ B^T)[k,o]
    (64 K=16 matmuls + in-place DVE add), keeping W_effT resident in SBUF.
  - Main GEMM per core: out[t,o] = sum_k xT[k,t]^T @ W_effT[k,o] with
    float32r (full-rate ~tf32 precision) matmuls, fp32 PSUM accumulation,
    bias fused into the PSUM->SBUF eviction (DVE tensor_add).
  - Host does layout only: shard/transpose x, transpose W/B, concat outputs.
"""
import sys

if "/opt/trn_rl_repo" not in sys.path:
    sys.path.insert(0, "/opt/trn_rl_repo")

import numpy as np

N_CORES = 8
B_, S, D_IN, D_OUT, R = 4, 4096, 2048, 2048, 16
SCALE = 1.0
T_CORE = (B_ * S) // N_CORES  # 2048 tokens per core
P = 128
N_TT = T_CORE // P   # 16 token tiles per core
N_KT = D_IN // P     # 16 contraction tiles
OC_W = 512           # psum bank width (fp32)
N_OC = D_OUT // OC_W # 4 output chunks

_CACHE = {}


def _build_program(n_iters: int = 1):
    import concourse.bacc as bacc
    import concourse.mybir as mybir
    import concourse.bass as bass
    from concourse import tile

    dt = mybir.dt
    nc = bacc.Bacc(None, target_bir_lowering=False, debug=False)

    x_in = nc.declare_dram_parameter(
        "x4", [N_TT, N_KT, P, P], dt.float32, isOutput=False
    )
    w_in = nc.declare_dram_parameter("wT", [D_IN, D_OUT], dt.float32, isOutput=False)
    a_in = nc.declare_dram_parameter("a", [R, D_IN], dt.float32, isOutput=False)
    bt_in = nc.declare_dram_parameter("bT", [R, D_OUT], dt.float32, isOutput=False)
    bias_in = nc.declare_dram_parameter("bias", [D_OUT], dt.float32, isOutput=False)
    out_ext = nc.declare_dram_parameter(
        "out", [T_CORE, D_OUT], dt.float32, isOutput=True
    )

    def body(tc, pools):
        const, wpool, xstage, xpool, opool, psumF, psumM = pools
        # --- small constants: A, B^T rounded to f32r via SWDGE cast-DMA ---
        a_sb = const.tile([R, D_IN], dt.float32r, tag="a_sb")
        bt_sb = const.tile([R, D_OUT], dt.float32r, tag="bt_sb")
        nc.gpsimd.dma_start(out=a_sb[:], in_=a_in[:])
        nc.gpsimd.dma_start(out=bt_sb[:], in_=bt_in[:])

        bias_sb = const.tile([P, D_OUT], dt.float32, tag="bias_sb")
        bias_ap = bias_in[:]
        bias_bcast = bass.AP(
            tensor=bias_ap.tensor,
            offset=bias_ap.offset,
            ap=[[0, P]] + list(bias_ap.ap),
        )
        nc.gpsimd.dma_start(out=bias_sb[:], in_=bias_bcast)

        # --- W_effT: cast-DMA W_baseT (rounds to f32r), fold LoRA on top ---
        w_tiles = []
        for kt in range(N_KT):
            wt = wpool.tile([P, D_OUT], dt.float32r, tag=f"w{kt}")
            w_tiles.append(wt)
            nc.gpsimd.dma_start(out=wt[:], in_=w_in[kt * P : (kt + 1) * P, :])
        for kt in range(N_KT):
            wt = w_tiles[kt]
            for oc in range(N_OC):
                ps = psumF.tile([P, OC_W], dt.float32, tag="psF")
                nc.tensor.matmul(
                    ps[:],
                    a_sb[:, kt * P : (kt + 1) * P],
                    bt_sb[:, oc * OC_W : (oc + 1) * OC_W],
                    start=True,
                    stop=True,
                )
                # W_eff = round_f32r(W_base + SCALE * (BA)^T); SCALE == 1.0
                nc.vector.tensor_add(
                    out=wt[:, oc * OC_W : (oc + 1) * OC_W],
                    in0=ps[:],
                    in1=wt[:, oc * OC_W : (oc + 1) * OC_W].bitcast(dt.float32),
                )

        # --- main GEMM over token tiles ---
        for tt in range(N_TT):
            xs = xstage.tile([P, N_KT, P], dt.float32, tag="xs")
            # DRAM x4[tt] is [kt, p(k), t]; SBUF wants [p(k), kt, t]
            nc.sync.dma_start(
                out=xs[:], in_=x_in[tt].rearrange("kt p t -> p kt t")
            )
            xt = xpool.tile([P, N_KT, P], dt.float32r, tag="xt")
            nc.vector.tensor_copy(xt[:], xs[:])
            for oc in range(N_OC):
                ps = psumM.tile([P, OC_W], dt.float32, tag="psM")
                for kt in range(N_KT):
                    nc.tensor.matmul(
                        ps[:],
                        xt[:, kt, :],
                        w_tiles[kt][:, oc * OC_W : (oc + 1) * OC_W],
                        start=(kt == 0),
                        stop=(kt == N_KT - 1),
                    )
                ot = opool.tile([P, OC_W], dt.float32, tag="ot")
                nc.vector.tensor_add(
                    out=ot[:],
                    in0=ps[:],
                    in1=bias_sb[:, oc * OC_W : (oc + 1) * OC_W],
                )
                nc.sync.dma_start(
                    out=out_ext[
                        tt * P : (tt + 1) * P, oc * OC_W : (oc + 1) * OC_W
                    ],
                    in_=ot[:],
                )

    with tile.TileContext(nc) as tc:
        with (
            tc.tile_pool(name="const", bufs=1) as const,
            tc.tile_pool(name="wpool", bufs=1) as wpool,
            tc.tile_pool(name="xstage", bufs=2) as xstage,
            tc.tile_pool(name="xpool", bufs=2) as xpool,
            tc.tile_pool(name="opool", bufs=4) as opool,
            tc.tile_pool(name="psumF", bufs=2, space="PSUM") as psumF,
            tc.tile_pool(name="psumM", bufs=4, space="PSUM") as psumM,
        ):
            pools = (const, wpool, xstage, xpool, opool, psumF, psumM)
            if n_iters == 1:
                body(tc, pools)
            else:
                with tc.For_i(0, n_iters, 1):
                    body(tc, pools)
    nc.compile()
    return nc


class _SpmdRunner:
    """Mirrors concourse.bass2jax.run_bass_via_pjrt but keeps the jitted
    executable alive so repeated calls don't recompile."""

    def __init__(self, nc, n_cores: int):
        import jax
        from jax.sharding import Mesh, PartitionSpec
        from jax.experimental.shard_map import shard_map
        import concourse.mybir as mybir
        from concourse.bass2jax import (
            _bass_exec_p,
            install_neuronx_cc_hook,
            partition_id_tensor,
        )

        install_neuronx_cc_hook()
        self.nc = nc
        self.n_cores = n_cores
        partition_name = (
            nc.partition_id_tensor.name if nc.partition_id_tensor else None
        )
        in_names, out_names, out_avals, zero_shapes = [], [], [], []
        for alloc in nc.m.functions[0].allocations:
            if not isinstance(alloc, mybir.MemoryLocationSet):
                continue
            name = alloc.memorylocations[0].name
            if alloc.kind == "ExternalInput":
                if name != partition_name:
                    in_names.append(name)
            elif alloc.kind == "ExternalOutput":
                shape = tuple(alloc.tensor_shape)
                dtype = mybir.dt.np(alloc.dtype)
                out_names.append(name)
                out_avals.append(jax.core.ShapedArray(shape, dtype))
                zero_shapes.append((shape, dtype))
        self.in_param_names = list(in_names)
        self.out_names = out_names
        self.out_avals = tuple(out_avals)
        self.zero_shapes = zero_shapes
        n_params = len(in_names)
        all_in_names = in_names + out_names
        if partition_name is not None:
            all_in_names.append(partition_name)
        n_outs = len(out_names)
        donate = tuple(range(n_params, n_params + n_outs))

        def _body(*args):
            operands = list(args)
            if partition_name is not None:
                operands.append(partition_id_tensor())
            outs = _bass_exec_p.bind(
                *operands,
                out_avals=self.out_avals,
                in_names=tuple(all_in_names),
                out_names=tuple(out_names),
                lowering_input_output_aliases=(),
                sim_require_finite=True,
                sim_require_nnan=True,
                nc=nc,
            )
            return tuple(outs)

        devices = jax.devices()[:n_cores]
        assert len(devices) == n_cores, (
            f"need {n_cores} neuron cores, found {len(jax.devices())}"
        )
        mesh = Mesh(np.asarray(devices), ("core",))
        in_specs = (PartitionSpec("core"),) * (n_params + n_outs)
        out_specs = (PartitionSpec("core"),) * n_outs
        self.sharded = jax.jit(
            shard_map(
                _body,
                mesh=mesh,
                in_specs=in_specs,
                out_specs=out_specs,
                check_rep=False,
            ),
            donate_argnums=donate,
            keep_unused=True,
        )

    def concat_inputs(self, in_maps):
        return [
            np.concatenate(
                [np.asarray(in_maps[c][n]) for c in range(self.n_cores)], axis=0
            )
            for n in self.in_param_names
        ]

    def _zeros(self):
        return [
            np.zeros((self.n_cores * s[0], *s[1:]), d)
            for (s, d) in self.zero_shapes
        ]

    def run_concat(self, concat_in):
        return self.sharded(*concat_in, *self._zeros())

    def run(self, in_maps):
        out_arrs = self.run_concat(self.concat_inputs(in_maps))
        res = []
        for c in range(self.n_cores):
            m = {}
            for i, name in enumerate(self.out_names):
                s = self.out_avals[i].shape
                m[name] = np.asarray(out_arrs[i]).reshape(self.n_cores, *s)[c]
            res.append(m)
        return res


def get_runner(n_iters: int = 1):
    key = ("runner", n_iters)
    if key not in _CACHE:
        nc = _build_program(n_iters=n_iters)
        _CACHE[key] = _SpmdRunner(nc, N_CORES)
    return _CACHE[key]


def make_in_maps(x, W_base, b_base, A, B):
    x2d = np.ascontiguousarray(x, dtype=np.float32).reshape(B_ * S, D_IN)
    wT = np.ascontiguousarray(W_base.T)
    bT = np.ascontiguousarray(B.T)
    a = np.ascontiguousarray(A)
    bias = np.ascontiguousarray(b_base)
    in_maps = []
    for c in range(N_CORES):
        xc = x2d[c * T_CORE : (c + 1) * T_CORE]  # [2048 t, 2048 k]
        # x4[tt, kt, p(k), t] = xc[tt*128 + t, kt*128 + p]
        x4 = np.ascontiguousarray(
            xc.reshape(N_TT, P, N_KT, P).transpose(0, 2, 3, 1)
        )
        in_maps.append({"x4": x4, "wT": wT, "a": a, "bT": bT, "bias": bias})
    return in_maps


def kernel(**inputs):
    x = inputs["x"]
    W_base = inputs["W_base"]
    b_base = inputs["b_base"]
    A = inputs["A"]
    B = inputs["B"]
    runner = get_runner()
    in_maps = make_in_maps(x, W_base, b_base, A, B)
    res = runner.run(in_maps)
    y2d = np.concatenate([res[c]["out"] for c in range(N_CORES)], axis=0)
    return np.ascontiguousarray(y2d.reshape(B_, S, D_OUT), dtype=np.float32)
